# revision 3
# baseline (speedup 1.0000x reference)
"""NT-Xent loss on 8 Trainium2 NeuronCores — fp8 DoubleRow version.

Strategy (per core c):
  - Host normalizes rows of z = concat(z_i, z_j) in fp64, scales by 16,
    quantizes to fp8 e4m3, and ships the TRANSPOSED, K-packed layout
    znt[p, h, j] = q[j, h*128 + p]  (shape [128, 2, 8192]) rotated by
    -1024*c columns so every core runs the identical program on "local
    rows 0..1023" (= local columns 0..1023 of znt). One NEFF, SPMD.
  - PSUM sim values are 256*sim (q = 16*zn, q.q' = 256 cos). Matmuls are
    fp8e4 with MatmulPerfMode.DoubleRow: one matmul per [128, 512]
    output covers the full K=256 contraction (lhsT [128,2,128],
    rhs [128,2,512]) at 0.5 cycles/row — 4x fewer PE cycles than the
    bf16 two-matmul pair.
  - ACT computes exp(2*sim) = Exp(psum * 1/128) in place on [128, 2048]
    PSUM windows; row sums are taken by DVE tensor_reduce (keeps the
    ACT engine free of READ_ACCUMULATOR overhead). Positive-pair sims
    are extracted pre-exp by a DVE identity-mask fused reduce.
  - Diag: sim_ii = |q_i|^2/256 = 1 + O(fp8 noise); the tail subtracts
    the constant e^2 (error ~2e-5 of the denominator).
  - Tail: denom = rowsum - e^2; ln(denom) via Newton seeded from the
    analytic scale (avoids an ACT table switch); per-core scalar
    partial via ones-matmul. Host sums the 8 partials / 8192.

fp8 error budget: e4m3 round-off sigma~3.6% per element; the 256-term
dot gives sim noise sigma~3e-3, which averages out in the row sums and
the mean over 8192 positives — observed loss error ~1e-4 relative.
"""

import os
import sys

sys.path.insert(0, "/opt/trn_rl_repo")
os.environ.setdefault("MYCRO_LOCAL_CACHE", "1")

import numpy as np

import concourse.bass as bass
import concourse.mybir as mybir
from concourse import bacc, tile
from concourse.bass_utils import run_bass_kernel_spmd

F32 = mybir.dt.float32
FP8 = mybir.dt.float8e4
AF = mybir.ActivationFunctionType
ALU = mybir.AluOpType
DROW = mybir.MatmulPerfMode.DoubleRow

N_CORES = 8
TWO_N = 8192
D = 256
P = 128
ROWS_PER_CORE = TWO_N // N_CORES  # 1024
M_CHUNKS = ROWS_PER_CORE // P     # 8 local row chunks
NCOL = 512                        # matmul free dim (one PSUM bank)
QCOL = 2048                       # ACT window = 4 banks
N_Q = TWO_N // QCOL               # 4
POS_OFF = TWO_N // 2              # 4096
TEMP_SCALE = 2.0                  # 1 / temperature
QSCALE = 16.0                     # host fp8 quant scale; psum = 256*sim
ACT_SCALE = TEMP_SCALE / (QSCALE * QSCALE)   # exp(psum/128)
POS_SCALE = TEMP_SCALE / (QSCALE * QSCALE)   # 2*pos = psum_pos/128

_NC_CACHE = {}


def _build_nc():
    nc = bacc.Bacc(
        "TRN2",
        target_bir_lowering=False,
        debug=False,
        enable_asserts=False,
        num_devices=N_CORES,
    )
    znt = nc.dram_tensor("znt", [P, 2, TWO_N], FP8, kind="ExternalInput")
    ident = nc.dram_tensor("ident", [P, P], F32, kind="ExternalInput")
    ones = nc.dram_tensor("ones", [P, 1], F32, kind="ExternalInput")
    out = nc.dram_tensor("partial", [1, 1], F32, kind="ExternalOutput")

    # column load groups: first two are small so the matmul pipeline can
    # start early; all on one ring so they complete in issue order
    GROUPS = [(0, 1024), (1024, 2048), (2048, 4096), (4096, 6144), (6144, TWO_N)]

    with tile.TileContext(nc) as tc:
        with (
            tc.tile_pool(name="big", bufs=1) as big,
            tc.tile_pool(name="work", bufs=2) as work,
        ):
            id_sb = big.tile([P, P], F32)
            nc.sync.dma_start(id_sb[:], ident[:])
            ones_sb = big.tile([P, 1], F32)
            nc.sync.dma_start(ones_sb[:], ones[:])

            znt_sb = big.tile([P, 2, TWO_N], FP8)
            for (c0, c1) in GROUPS:
                nc.gpsimd.dma_start(znt_sb[:, :, c0:c1], znt[:, :, c0:c1])

            # preload the Exp activation table while DMAs stream
            tbl = big.tile([P, 1], F32)
            nc.scalar.activation(tbl[:], ones_sb[:], AF.Exp)

            sums = big.tile([P, M_CHUNKS * N_Q], F32)
            pos = big.tile([P, M_CHUNKS], F32)

            with tc.tile_pool(name="psum", bufs=2, space="PSUM") as psum_pool:
                # warm-up matmuls: ramp the PE p-state on group-0 data
                ptw = psum_pool.tile([P, QCOL], F32, tag="sim")
                for j in range(8):
                    nc.tensor.matmul(
                        ptw[:, (j % 4) * NCOL:(j % 4) * NCOL + NCOL],
                        znt_sb[:, :, 0:P],
                        znt_sb[:, :, (j % 2) * NCOL:(j % 2) * NCOL + NCOL],
                        start=True, stop=True,
                        perf_mode=DROW, skip_group_check=True)

                for q in range(N_Q):
                    for m in range(M_CHUNKS):
                        pt = psum_pool.tile([P, QCOL], F32, tag="sim")
                        lhsT = znt_sb[:, :, m * P:(m + 1) * P]
                        for nn in range(QCOL // NCOL):
                            col = q * QCOL + nn * NCOL
                            nc.tensor.matmul(
                                pt[:, nn * NCOL:(nn + 1) * NCOL],
                                lhsT,
                                znt_sb[:, :, col:col + NCOL],
                                start=True, stop=True,
                                perf_mode=DROW)
                        pcol = POS_OFF + m * P
                        if q * QCOL <= pcol < (q + 1) * QCOL:
                            off = pcol - q * QCOL
                            scr = work.tile([P, P], F32, tag="extr")
                            nc.vector.scalar_tensor_tensor(
                                out=scr[:], in0=pt[:, off:off + P],
                                scalar=1.0, in1=id_sb[:],
                                op0=ALU.mult, op1=ALU.mult,
                                accum_out=pos[:, m:m + 1])
                        nc.scalar.activation(pt[:], pt[:], AF.Exp,
                                             scale=ACT_SCALE)
                        col_ix = m * N_Q + q
                        nc.vector.tensor_reduce(
                            sums[:, col_ix:col_ix + 1], pt[:],
                            axis=mybir.AxisListType.X, op=ALU.add)

            # ---- tail: per-core partial loss ----
            stot = big.tile([P, M_CHUNKS], F32)
            nc.vector.tensor_reduce(
                stot[:],
                sums[:].rearrange("p (m q) -> p m q", q=N_Q),
                axis=mybir.AxisListType.X,
                op=ALU.add,
            )
            denom = big.tile([P, M_CHUNKS], F32)
            nc.vector.tensor_scalar_add(denom[:], stot[:],
                                        -float(np.exp(TEMP_SCALE)))
            # ln(denom) via Newton (keeps the ACT table pinned to Exp):
            # y <- y + denom*exp(-y) - 1, seeded with the analytic denom
            # scale; one DVE step with constant exp(-y0), one with a real
            # exp.
            y0 = float(np.log((TWO_N - 1) * np.exp(TEMP_SCALE ** 2 / (2 * D))))
            e1 = float(np.exp(-y0))
            y1 = big.tile([P, M_CHUNKS], F32)
            nc.vector.tensor_scalar(y1[:], denom[:], e1, y0 - 1.0,
                                    ALU.mult, ALU.add)
            e2t = big.tile([P, M_CHUNKS], F32)
            nc.scalar.activation(e2t[:], y1[:], AF.Exp, scale=-1.0)
            tprod = big.tile([P, M_CHUNKS], F32)
            nc.vector.tensor_mul(tprod[:], e2t[:], denom[:])
            lnd = big.tile([P, M_CHUNKS], F32)
            nc.vector.scalar_tensor_tensor(
                out=lnd[:], in0=tprod[:], scalar=-1.0, in1=y1[:],
                op0=ALU.add, op1=ALU.add)
            term = big.tile([P, M_CHUNKS], F32)
            tsum = big.tile([P, 1], F32)
            nc.vector.scalar_tensor_tensor(
                out=term[:], in0=pos[:], scalar=-POS_SCALE, in1=lnd[:],
                op0=ALU.mult, op1=ALU.add, accum_out=tsum[:])
            with tc.tile_pool(name="psum2", bufs=1, space="PSUM") as pp2:
                pfin = pp2.tile([1, 1], F32)
                nc.tensor.matmul(pfin[:], ones_sb[:], tsum[:],
                                 start=True, stop=True)
                res = big.tile([1, 1], F32)
                nc.vector.tensor_copy(res[:], pfin[:])
                nc.sync.dma_start(out[:], res[:])

    nc.compile()
    return nc


def _get_nc():
    if "nc" not in _NC_CACHE:
        _NC_CACHE["nc"] = _build_nc()
    return _NC_CACHE["nc"]


def _prepare_in_maps(z_i, z_j):
    import ml_dtypes

    z = np.concatenate(
        [np.asarray(z_i, np.float64), np.asarray(z_j, np.float64)], axis=0
    )
    zn = z / np.linalg.norm(z, axis=1, keepdims=True)
    q = (zn * QSCALE).astype(np.float32).astype(ml_dtypes.float8_e4m3)
    # znt[p, h, j] = q[j, h*128 + p]
    znt = np.ascontiguousarray(
        q.T.reshape(2, P, TWO_N).transpose(1, 0, 2))
    ident = np.eye(P, dtype=np.float32)
    onesv = np.ones((P, 1), dtype=np.float32)
    in_maps = []
    for c in range(N_CORES):
        zc = np.roll(znt, -ROWS_PER_CORE * c, axis=2)
        in_maps.append(
            {"znt": np.ascontiguousarray(zc), "ident": ident, "ones": onesv})
    return in_maps


def kernel(z_i, z_j):
    nc = _get_nc()
    in_maps = _prepare_in_maps(z_i, z_j)
    res = run_bass_kernel_spmd(nc, in_maps, core_ids=list(range(N_CORES)))
    total = 0.0
    for c in range(N_CORES):
        total += float(res.results[c]["partial"][0, 0])
    loss = total / float(TWO_N)
    return np.float32(loss)


if __name__ == "__main__":
    rng = np.random.default_rng(0)
    z_i = rng.standard_normal((4096, 256), dtype=np.float32)
    z_j = rng.standard_normal((4096, 256), dtype=np.float32)
    print("loss:", kernel(z_i, z_j))


# revision 6
# speedup vs baseline: 1.7877x; 1.7877x over previous
"""NT-Xent loss on 8 Trainium2 NeuronCores — fp8 + symmetric 5/8 scheme.

Math: z = concat(z_i, z_j) [8192, 256]; E = exp(2*cos_sim(z)) is
symmetric, so row sums = column sums. Blocked over 8 superblocks of
1024 rows, core c owns rows [1024c, 1024c+1024) and computes sim
against column superblocks d = 0..4 (its own + the next four, mod 8).
d in {1,2,3} blocks are mirrored via COLUMN sums (row sums of the
uncomputed transpose block); the d=4 block is computed by both members
of each (c, c+4) pair, so no mirror is needed there. Every global row
then receives exactly 8 superblock contributions: 5 direct row-sum
partials from its owner core + 3 column-sum partials from cores
c-1, c-2, c-3. The host assembles denom = sum - e^2, takes ln in
fp64, and averages — O(2N) work.

Per core (identical SPMD program on column-rotated inputs):
  - Host normalizes rows in fp64, scales by 16, quantizes to fp8 e4m3,
    ships the transposed K-packed layout znt[p, h, j] = q[j, h*128+p]
    ([128, 2, 5120], rotated by -1024c cols: local cols 0:5120 =
    superblocks c..c+4, local rows = local cols 0:1024).
  - Sim matmuls: fp8 DoubleRow, one matmul per [128, 512] output
    covering the full K=256 contraction. PSUM = 256*sim.
  - ACT exps [128, 2048] windows (scale 1/128) with fused row-sum
    accumulation. Windows over cols 0:4096 write exp as bf16 to SBUF
    (esb) for the colsum path; the [4096:5120) window is exp'd in
    place and carries the positive-pair diagonal (extracted pre-exp by
    a DVE identity-mask reduce).
  - Colsums over cols 1024:4096: DVE bf16 tree-sum over the 8 m-chunks
    (2x mode), then one ones-matmul per 512-col block -> [1, 512] PSUM
    -> colsum_sb. PE cost ~2 us, DVE ~12 us (otherwise idle).
  - Outputs per core: rowsums [128, 8], colsums [1, 3072], posv
    [128, 8] (scaled sim, x1/128 on host).

fp8 error budget: e4m3 sigma~3.6%/elem -> sim noise sigma~3e-3 which
averages out in row sums and the 8192-row mean; observed ~1e-5 rel.
"""

import os
import sys

sys.path.insert(0, "/opt/trn_rl_repo")
os.environ.setdefault("MYCRO_LOCAL_CACHE", "1")

import numpy as np

import concourse.bass as bass
import concourse.mybir as mybir
from concourse import bacc, tile
from concourse.bass_utils import run_bass_kernel_spmd

F32 = mybir.dt.float32
BF16 = mybir.dt.bfloat16
FP8 = mybir.dt.float8e4
AF = mybir.ActivationFunctionType
ALU = mybir.AluOpType
DROW = mybir.MatmulPerfMode.DoubleRow

N_CORES = 8
TWO_N = 8192
D = 256
P = 128
ROWS_PER_CORE = TWO_N // N_CORES  # 1024
M_CHUNKS = ROWS_PER_CORE // P     # 8 local row chunks
NCOL = 512                        # matmul free dim (one PSUM bank)
QCOL = 2048                       # ACT window for q=0,1
LCOLS = 5 * ROWS_PER_CORE         # 5120 local cols (superblocks d=0..4)
CS_LO, CS_HI = 1024, 4096         # colsum region (d=1..3)
POS_Q0 = 4096                     # start of the d=4 window
TEMP_SCALE = 2.0
QSCALE = 16.0                     # host fp8 quant scale; psum = 256*sim
ACT_SCALE = TEMP_SCALE / (QSCALE * QSCALE)   # exp(psum/128)
POS_SCALE = TEMP_SCALE / (QSCALE * QSCALE)

_NC_CACHE = {}


def _build_nc():
    nc = bacc.Bacc(
        "TRN2",
        target_bir_lowering=False,
        debug=False,
        enable_asserts=False,
        num_devices=N_CORES,
    )
    znt = nc.dram_tensor("znt", [P, 2, LCOLS], FP8, kind="ExternalInput")
    ident = nc.dram_tensor("ident", [P, P], F32, kind="ExternalInput")
    onesb = nc.dram_tensor("onesb", [P, 1], BF16, kind="ExternalInput")
    rowsums_d = nc.dram_tensor("rowsums", [P, M_CHUNKS], F32,
                               kind="ExternalOutput")
    colsums_d = nc.dram_tensor("colsums", [1, CS_HI - CS_LO], F32,
                               kind="ExternalOutput")
    posv_d = nc.dram_tensor("posv", [P, M_CHUNKS], F32,
                            kind="ExternalOutput")

    N_WIN = 3  # rowsum windows per m: [0:2048), [2048:4096), [4096:5120)

    with tile.TileContext(nc) as tc:
        with (
            tc.tile_pool(name="big", bufs=1) as big,
            tc.tile_pool(name="esbp", bufs=1) as esbp,
            tc.tile_pool(name="work", bufs=2) as work,
        ):
            id_sb = big.tile([P, P], F32)
            nc.sync.dma_start(id_sb[:], ident[:])
            ones_sb = big.tile([P, 1], BF16)
            nc.sync.dma_start(ones_sb[:], onesb[:])

            znt_sb = big.tile([P, 2, LCOLS], FP8)
            for g in range(5):
                c0, c1 = g * 1024, (g + 1) * 1024
                nc.gpsimd.dma_start(znt_sb[:, :, c0:c1], znt[:, :, c0:c1])

            # preload the Exp table while DMAs stream
            tbl = big.tile([P, 1], F32)
            nc.scalar.activation(tbl[:], id_sb[:, 0:1], AF.Exp)

            sums = big.tile([P, M_CHUNKS * N_WIN], F32)
            pos = big.tile([P, M_CHUNKS], F32)
            # exp windows for q=0,1 (cols 0:4096), bf16, m-major
            esb = [esbp.tile([P, M_CHUNKS, QCOL], BF16, tag=f"esb{q}",
                             name=f"esb{q}")
                   for q in range(2)]
            esum = [esbp.tile([P, 1024], BF16, tag="esum0", name="esum0"),
                    esbp.tile([P, QCOL], BF16, tag="esum1", name="esum1")]
            colsum_sb = big.tile([1, CS_HI - CS_LO], F32)

            def tree_sum(dst, src_m, sl, width, q):
                """dst[:, :width] = sum over m of src_m[:, m, sl] (bf16)."""
                lvl = [src_m[:, m, sl] for m in range(M_CHUNKS)]
                depth = 0
                while len(lvl) > 2:
                    nxt = []
                    for k in range(0, len(lvl), 2):
                        t = work.tile([P, width], BF16,
                                      tag=f"ts{q}_{depth}_{k}", bufs=1)
                        nc.vector.tensor_tensor(t[:], lvl[k], lvl[k + 1],
                                                ALU.add)
                        nxt.append(t[:])
                    lvl = nxt
                    depth += 1
                nc.vector.tensor_tensor(dst, lvl[0], lvl[1], ALU.add)

            with tc.tile_pool(name="psum", bufs=2, space="PSUM") as psum_pool:
                # warm-up matmuls: ramp the PE p-state on group-0 data
                ptw = psum_pool.tile([P, QCOL], F32, tag="sim")
                for j in range(8):
                    nc.tensor.matmul(
                        ptw[:, (j % 4) * NCOL:(j % 4) * NCOL + NCOL],
                        znt_sb[:, :, 0:P],
                        znt_sb[:, :, (j % 2) * NCOL:(j % 2) * NCOL + NCOL],
                        start=True, stop=True,
                        perf_mode=DROW, skip_group_check=True)

                for q in range(2):      # cols [0:2048), [2048:4096) -> esb
                    for m in range(M_CHUNKS):
                        pt = psum_pool.tile([P, QCOL], F32, tag="sim")
                        lhsT = znt_sb[:, :, m * P:(m + 1) * P]
                        for nn in range(QCOL // NCOL):
                            col = q * QCOL + nn * NCOL
                            nc.tensor.matmul(
                                pt[:, nn * NCOL:(nn + 1) * NCOL],
                                lhsT,
                                znt_sb[:, :, col:col + NCOL],
                                start=True, stop=True, perf_mode=DROW)
                        col_ix = m * N_WIN + q
                        nc.scalar.activation(
                            esb[q][:, m, :], pt[:], AF.Exp, scale=ACT_SCALE,
                            accum_out=sums[:, col_ix:col_ix + 1])
                    # m-tree colsum partials for this q while q+1 computes
                    if q == 0:
                        tree_sum(esum[0][:], esb[0], slice(1024, 2048),
                                 1024, 0)
                    else:
                        tree_sum(esum[1][:], esb[1], slice(0, QCOL),
                                 QCOL, 1)

                for m in range(M_CHUNKS):   # d=4 window, in place + pos
                    ptf = psum_pool.tile([P, QCOL], F32, tag="sim")
                    pt = ptf[:, 0:1024]
                    lhsT = znt_sb[:, :, m * P:(m + 1) * P]
                    for nn in range(2):
                        col = POS_Q0 + nn * NCOL
                        nc.tensor.matmul(
                            pt[:, nn * NCOL:(nn + 1) * NCOL],
                            lhsT,
                            znt_sb[:, :, col:col + NCOL],
                            start=True, stop=True, perf_mode=DROW)
                    off = m * P
                    scr = work.tile([P, P], F32, tag="extr")
                    nc.vector.scalar_tensor_tensor(
                        out=scr[:], in0=pt[:, off:off + P],
                        scalar=1.0, in1=id_sb[:],
                        op0=ALU.mult, op1=ALU.mult,
                        accum_out=pos[:, m:m + 1])
                    col_ix = m * N_WIN + 2
                    nc.scalar.activation(
                        pt[:], pt[:], AF.Exp, scale=ACT_SCALE,
                        accum_out=sums[:, col_ix:col_ix + 1])

            # colsum matmuls: [1, 512] = ones.T @ esum block, then copy out
            with tc.tile_pool(name="psum2", bufs=2, space="PSUM") as pp2:
                for b in range(6):
                    c0 = b * NCOL            # offset within cols 1024:4096
                    if c0 < 1024:
                        src = esum[0][:, c0:c0 + NCOL]
                    else:
                        src = esum[1][:, c0 - 1024:c0 - 1024 + NCOL]
                    pc = pp2.tile([1, NCOL], F32, tag="cs")
                    nc.tensor.matmul(pc[:], ones_sb[:], src,
                                     start=True, stop=True)
                    nc.vector.tensor_copy(colsum_sb[:, c0:c0 + NCOL], pc[:])

            # rowsum partials: sum the 3 windows per m
            stot = big.tile([P, M_CHUNKS], F32)
            nc.vector.tensor_reduce(
                stot[:],
                sums[:].rearrange("p (m q) -> p m q", q=N_WIN),
                axis=mybir.AxisListType.X,
                op=ALU.add,
            )
            nc.sync.dma_start(rowsums_d[:], stot[:])
            nc.sync.dma_start(colsums_d[:], colsum_sb[:])
            nc.sync.dma_start(posv_d[:], pos[:])

    nc.compile()
    return nc


def _get_nc():
    if "nc" not in _NC_CACHE:
        _NC_CACHE["nc"] = _build_nc()
    return _NC_CACHE["nc"]


def _prepare_in_maps(z_i, z_j):
    import ml_dtypes

    z = np.concatenate(
        [np.asarray(z_i, np.float64), np.asarray(z_j, np.float64)], axis=0
    )
    zn = z / np.linalg.norm(z, axis=1, keepdims=True)
    q = (zn * QSCALE).astype(np.float32).astype(ml_dtypes.float8_e4m3)
    # znt[p, h, j] = q[j, h*128 + p]
    znt = np.ascontiguousarray(q.T.reshape(2, P, TWO_N).transpose(1, 0, 2))
    ident = np.eye(P, dtype=np.float32)
    onesb = np.ones((P, 1), dtype=ml_dtypes.bfloat16)
    in_maps = []
    for c in range(N_CORES):
        zc = np.roll(znt, -ROWS_PER_CORE * c, axis=2)[:, :, :LCOLS]
        in_maps.append(
            {"znt": np.ascontiguousarray(zc), "ident": ident,
             "onesb": onesb})
    return in_maps


def _combine(results):
    """Assemble the loss from per-core rowsum/colsum/pos partials."""
    total = np.zeros(TWO_N, dtype=np.float64)
    posg = np.zeros(TWO_N, dtype=np.float64)
    for c in range(N_CORES):
        r0 = c * ROWS_PER_CORE
        rs = np.asarray(results[c]["rowsums"], np.float64)  # [128, 8]
        pv = np.asarray(results[c]["posv"], np.float64)
        for m in range(M_CHUNKS):
            gsl = slice(r0 + m * P, r0 + (m + 1) * P)
            total[gsl] += rs[:, m]
            posg[gsl] = pv[:, m]
        cs = np.asarray(results[c]["colsums"], np.float64).ravel()  # [3072]
        gidx = (r0 + CS_LO + np.arange(CS_HI - CS_LO)) % TWO_N
        np.add.at(total, gidx, cs)
    denom = total - np.exp(TEMP_SCALE)
    terms = np.log(denom) - POS_SCALE * posg
    return float(terms.mean())


def kernel(z_i, z_j):
    nc = _get_nc()
    in_maps = _prepare_in_maps(z_i, z_j)
    res = run_bass_kernel_spmd(nc, in_maps, core_ids=list(range(N_CORES)))
    return np.float32(_combine(res.results))


if __name__ == "__main__":
    rng = np.random.default_rng(0)
    z_i = rng.standard_normal((4096, 256), dtype=np.float32)
    z_j = rng.standard_normal((4096, 256), dtype=np.float32)
    print("loss:", kernel(z_i, z_j))


# revision 12
# speedup vs baseline: 1.8358x; 1.0269x over previous
"""NT-Xent loss on 8 Trainium2 NeuronCores — fp8 + symmetric 5/8 scheme.

Math: z = concat(z_i, z_j) [8192, 256]; E = exp(2*cos_sim(z)) is
symmetric, so row sums = column sums. Blocked over 8 superblocks of
1024 rows, core c owns rows [1024c, 1024c+1024) and computes sim
against column superblocks d = 0..4 (its own + the next four, mod 8).
d in {1,2,3} blocks are mirrored via COLUMN sums (row sums of the
uncomputed transpose block); the d=4 block is computed by both members
of each (c, c+4) pair, so no mirror is needed there. Every global row
then receives exactly 8 superblock contributions: 5 direct row-sum
partials from its owner core + 3 column-sum partials from cores
c-1, c-2, c-3. The host assembles denom = sum - e^2, takes ln in
fp64, and averages — O(2N) work.

Per core (identical SPMD program on column-rotated inputs):
  - Host normalizes rows in fp64, scales by 16, quantizes to fp8 e4m3,
    ships the transposed K-packed layout znt[p, h, j] = q[j, h*128+p]
    ([128, 2, 5120], rotated by -1024c cols: local cols 0:5120 =
    superblocks c..c+4, local rows = local cols 0:1024).
  - Sim matmuls: fp8 DoubleRow, one matmul per [128, 512] output
    covering the full K=256 contraction. PSUM = 256*sim.
  - ACT exps [128, 2048] windows (scale 1/128) with fused row-sum
    accumulation. Windows over cols 0:4096 write exp as bf16 to SBUF
    (esb) for the colsum path; the [4096:5120) window is exp'd in
    place and carries the positive-pair diagonal (extracted pre-exp by
    a DVE identity-mask reduce).
  - Colsums over cols 1024:4096: DVE bf16 tree-sum over the 8 m-chunks
    (2x mode), then one ones-matmul per 512-col block -> [1, 512] PSUM
    -> colsum_sb. PE cost ~2 us, DVE ~12 us (otherwise idle).
  - Outputs per core: rowsums [128, 8], colsums [1, 3072], posv
    [128, 8] (scaled sim, x1/128 on host).

fp8 error budget: e4m3 sigma~3.6%/elem -> sim noise sigma~3e-3 which
averages out in row sums and the 8192-row mean; observed ~1e-5 rel.
"""

import os
import sys

sys.path.insert(0, "/opt/trn_rl_repo")
os.environ.setdefault("MYCRO_LOCAL_CACHE", "1")

import numpy as np

import concourse.bass as bass
import concourse.mybir as mybir
from concourse import bacc, tile
from concourse.bass_utils import run_bass_kernel_spmd

F32 = mybir.dt.float32
BF16 = mybir.dt.bfloat16
FP8 = mybir.dt.float8e4
AF = mybir.ActivationFunctionType
ALU = mybir.AluOpType
DROW = mybir.MatmulPerfMode.DoubleRow

N_CORES = 8
TWO_N = 8192
D = 256
P = 128
ROWS_PER_CORE = TWO_N // N_CORES  # 1024
M_CHUNKS = ROWS_PER_CORE // P     # 8 local row chunks
NCOL = 512                        # matmul free dim (one PSUM bank)
QCOL = 2048                       # ACT window for q=0,1
LCOLS = 5 * ROWS_PER_CORE         # 5120 local cols (superblocks d=0..4)
CS_LO, CS_HI = 1024, 4096         # colsum region (d=1..3)
POS_Q0 = 4096                     # start of the d=4 window
TEMP_SCALE = 2.0
QSCALE = 16.0                     # host fp8 quant scale; psum = 256*sim
ACT_SCALE = TEMP_SCALE / (QSCALE * QSCALE)   # exp(psum/128)
POS_SCALE = TEMP_SCALE / (QSCALE * QSCALE)

_NC_CACHE = {}


def _build_nc():
    nc = bacc.Bacc(
        "TRN2",
        target_bir_lowering=False,
        debug=False,
        enable_asserts=False,
        num_devices=N_CORES,
    )
    znt = nc.dram_tensor("znt", [P, 2, LCOLS], FP8, kind="ExternalInput")
    ident = nc.dram_tensor("ident", [P, P], F32, kind="ExternalInput")
    onesb = nc.dram_tensor("onesb", [P, 1], BF16, kind="ExternalInput")
    rowsums_d = nc.dram_tensor("rowsums", [P, M_CHUNKS], F32,
                               kind="ExternalOutput")
    colsums_d = nc.dram_tensor("colsums", [1, CS_HI - CS_LO], F32,
                               kind="ExternalOutput")
    posv_d = nc.dram_tensor("posv", [P, M_CHUNKS], F32,
                            kind="ExternalOutput")

    N_WIN = 3  # rowsum windows per m: [0:2048), [2048:4096), [4096:5120)

    with tile.TileContext(nc) as tc:
        with (
            tc.tile_pool(name="big", bufs=1) as big,
            tc.tile_pool(name="esbp", bufs=1) as esbp,
            tc.tile_pool(name="work", bufs=2) as work,
        ):
            id_sb = big.tile([P, P], F32)
            nc.sync.dma_start(id_sb[:], ident[:])
            ones_sb = big.tile([P, 1], BF16)
            nc.sync.dma_start(ones_sb[:], onesb[:])

            znt_sb = big.tile([P, 2, LCOLS], FP8)
            for g in range(5):
                c0, c1 = g * 1024, (g + 1) * 1024
                nc.gpsimd.dma_start(znt_sb[:, :, c0:c1], znt[:, :, c0:c1])

            # preload the Exp table while DMAs stream
            tbl = big.tile([P, 1], F32)
            nc.scalar.activation(tbl[:], id_sb[:, 0:1], AF.Exp)

            sums = big.tile([P, M_CHUNKS * N_WIN], F32)
            pos = big.tile([P, M_CHUNKS], F32)
            # exp windows for q=0,1 (cols 0:4096), bf16, m-major
            esb = [esbp.tile([P, M_CHUNKS, QCOL], BF16, tag=f"esb{q}",
                             name=f"esb{q}")
                   for q in range(2)]
            esum = [esbp.tile([P, 1024], BF16, tag="esum0", name="esum0"),
                    esbp.tile([P, QCOL], BF16, tag="esum1", name="esum1")]
            colsum_sb = big.tile([1, CS_HI - CS_LO], F32)

            def tree_sum(dst, src_m, sl, width, q):
                """dst[:, :width] = sum over m of src_m[:, m, sl] (bf16)."""
                lvl = [src_m[:, m, sl] for m in range(M_CHUNKS)]
                depth = 0
                while len(lvl) > 2:
                    nxt = []
                    for k in range(0, len(lvl), 2):
                        t = work.tile([P, width], BF16,
                                      tag=f"ts{q}_{depth}_{k}", bufs=1)
                        nc.vector.tensor_tensor(t[:], lvl[k], lvl[k + 1],
                                                ALU.add)
                        nxt.append(t[:])
                    lvl = nxt
                    depth += 1
                nc.vector.tensor_tensor(dst, lvl[0], lvl[1], ALU.add)

            with tc.tile_pool(name="psum", bufs=2, space="PSUM") as psum_pool:
                # warm-up matmuls: ramp the PE p-state on group-0 data
                ptw = psum_pool.tile([P, QCOL], F32, tag="sim")
                for j in range(2):
                    nc.tensor.matmul(
                        ptw[:, j * NCOL:(j + 1) * NCOL],
                        znt_sb[:, :, 0:P],
                        znt_sb[:, :, j * NCOL:(j + 1) * NCOL],
                        start=True, stop=True,
                        perf_mode=DROW, skip_group_check=True)

                for q in range(2):      # cols [0:2048), [2048:4096) -> esb
                    for m in range(M_CHUNKS):
                        pt = psum_pool.tile([P, QCOL], F32, tag="sim")
                        lhsT = znt_sb[:, :, m * P:(m + 1) * P]
                        for nn in range(QCOL // NCOL):
                            col = q * QCOL + nn * NCOL
                            nc.tensor.matmul(
                                pt[:, nn * NCOL:(nn + 1) * NCOL],
                                lhsT,
                                znt_sb[:, :, col:col + NCOL],
                                start=True, stop=True, perf_mode=DROW)
                        col_ix = m * N_WIN + q
                        nc.scalar.activation(
                            esb[q][:, m, :], pt[:], AF.Exp, scale=ACT_SCALE,
                            accum_out=sums[:, col_ix:col_ix + 1])
                    # m-tree colsum partials for this q while q+1 computes
                    if q == 0:
                        tree_sum(esum[0][:], esb[0], slice(1024, 2048),
                                 1024, 0)
                    else:
                        tree_sum(esum[1][:], esb[1], slice(0, QCOL),
                                 QCOL, 1)

            # d=4 windows + colsum matmuls share a fresh 6-bank pool so the
            # colsum work overlaps the d4 phase instead of trailing it
            with tc.tile_pool(name="psum2", bufs=2, space="PSUM") as pp2:
                def colsum_block(b):
                    c0 = b * NCOL            # offset within cols 1024:4096
                    if c0 < 1024:
                        src = esum[0][:, c0:c0 + NCOL]
                    else:
                        src = esum[1][:, c0 - 1024:c0 - 1024 + NCOL]
                    pc = pp2.tile([1, NCOL], F32, tag="cs", name="pc")
                    nc.tensor.matmul(pc[:], ones_sb[:], src,
                                     start=True, stop=True)
                    nc.vector.tensor_copy(colsum_sb[:, c0:c0 + NCOL], pc[:])

                for m in range(M_CHUNKS):   # d=4 window, in place + pos
                    pt = pp2.tile([P, 1024], F32, tag="d4")
                    lhsT = znt_sb[:, :, m * P:(m + 1) * P]
                    for nn in range(2):
                        col = POS_Q0 + nn * NCOL
                        nc.tensor.matmul(
                            pt[:, nn * NCOL:(nn + 1) * NCOL],
                            lhsT,
                            znt_sb[:, :, col:col + NCOL],
                            start=True, stop=True, perf_mode=DROW)
                    if m < 2:
                        colsum_block(m)      # esum[0] blocks, ready early
                    off = m * P
                    scr = work.tile([P, P], F32, tag="extr")
                    nc.vector.scalar_tensor_tensor(
                        out=scr[:], in0=pt[:, off:off + P],
                        scalar=1.0, in1=id_sb[:],
                        op0=ALU.mult, op1=ALU.mult,
                        accum_out=pos[:, m:m + 1])
                    col_ix = m * N_WIN + 2
                    nc.scalar.activation(
                        pt[:], pt[:], AF.Exp, scale=ACT_SCALE,
                        accum_out=sums[:, col_ix:col_ix + 1])
                for b in range(2, 6):        # esum[1] blocks
                    colsum_block(b)
                nc.sync.dma_start(colsums_d[:], colsum_sb[:])

            # rowsum partials: sum the 3 windows per m
            stot = big.tile([P, M_CHUNKS], F32)
            nc.vector.tensor_reduce(
                stot[:],
                sums[:].rearrange("p (m q) -> p m q", q=N_WIN),
                axis=mybir.AxisListType.X,
                op=ALU.add,
            )
            nc.sync.dma_start(rowsums_d[:], stot[:])
            nc.sync.dma_start(posv_d[:], pos[:])

    nc.compile()
    return nc


def _get_nc():
    if "nc" not in _NC_CACHE:
        _NC_CACHE["nc"] = _build_nc()
    return _NC_CACHE["nc"]


def _prepare_in_maps(z_i, z_j):
    import ml_dtypes

    z = np.concatenate(
        [np.asarray(z_i, np.float64), np.asarray(z_j, np.float64)], axis=0
    )
    zn = z / np.linalg.norm(z, axis=1, keepdims=True)
    q = (zn * QSCALE).astype(np.float32).astype(ml_dtypes.float8_e4m3)
    # znt[p, h, j] = q[j, h*128 + p]
    znt = np.ascontiguousarray(q.T.reshape(2, P, TWO_N).transpose(1, 0, 2))
    ident = np.eye(P, dtype=np.float32)
    onesb = np.ones((P, 1), dtype=ml_dtypes.bfloat16)
    in_maps = []
    for c in range(N_CORES):
        zc = np.roll(znt, -ROWS_PER_CORE * c, axis=2)[:, :, :LCOLS]
        in_maps.append(
            {"znt": np.ascontiguousarray(zc), "ident": ident,
             "onesb": onesb})
    return in_maps


def _combine(results):
    """Assemble the loss from per-core rowsum/colsum/pos partials."""
    total = np.zeros(TWO_N, dtype=np.float64)
    posg = np.zeros(TWO_N, dtype=np.float64)
    for c in range(N_CORES):
        r0 = c * ROWS_PER_CORE
        rs = np.asarray(results[c]["rowsums"], np.float64)  # [128, 8]
        pv = np.asarray(results[c]["posv"], np.float64)
        for m in range(M_CHUNKS):
            gsl = slice(r0 + m * P, r0 + (m + 1) * P)
            total[gsl] += rs[:, m]
            posg[gsl] = pv[:, m]
        cs = np.asarray(results[c]["colsums"], np.float64).ravel()  # [3072]
        gidx = (r0 + CS_LO + np.arange(CS_HI - CS_LO)) % TWO_N
        np.add.at(total, gidx, cs)
    denom = total - np.exp(TEMP_SCALE)
    terms = np.log(denom) - POS_SCALE * posg
    return float(terms.mean())


def kernel(z_i, z_j):
    nc = _get_nc()
    in_maps = _prepare_in_maps(z_i, z_j)
    res = run_bass_kernel_spmd(nc, in_maps, core_ids=list(range(N_CORES)))
    return np.float32(_combine(res.results))


if __name__ == "__main__":
    rng = np.random.default_rng(0)
    z_i = rng.standard_normal((4096, 256), dtype=np.float32)
    z_j = rng.standard_normal((4096, 256), dtype=np.float32)
    print("loss:", kernel(z_i, z_j))
